# revision 33
# baseline (speedup 1.0000x reference)
"""Trainium2 Bass kernel for nn_AttentionPool1d (sliding-window self-attention pool).

Math (faithful to the reference):
    xp = pad(x, 4 each side on T)                    # [B, D, S], S = T + 8
    Y[:, s]  = Wq @ xp[:, s] + bq                    # Q and K share this projection
    Z[:, s]  = Wo @ xp[:, s]                         # V is raw xp; Wo commutes with the
                                                     #   attention average (linearity)
    energy[t, w] = Y[:, t+4] . Y[:, t+w] / (sqrt(D) * 1.5)
    attn = softmax_w(energy)
    out[:, t] = sum_w attn[t, w] * Z[:, t+w] + bo    # since sum_w attn = 1

Distribution: pure data-parallel over batch, 2 batches per NeuronCore, 8 cores.

Precision plan (validated by emulation vs the fp32 reference, seed-0 inputs):
    - The Q/K path tolerates fp8: Y-projection and the gram run as fp8e4m3
      DoubleRow matmuls (256-deep contraction per instruction). Host pre-scales
      x by 8 (fp16 and fp8 copies both carry 8x) and Wq by 64 (fp8 carries
      64Wq) to keep fp8 operands out of the subnormal range; the 1/64 rescale
      + 8bq bias fold into the PSUM evacuation, which emits Y8 = 8Y in fp8.
      The gram PSUM is then 64*energy*sqrt(D)*1.5, folded into the Exp scale.
    - The value path does NOT tolerate fp8 (max-rel-err 4e-2 > 2e-2 budget):
      Z stays fp16. ZT carries 8Z; the F = ZT^T @ attn PSUM is 8*out, and the
      1/8 + bo fold into the F evacuation. Output is written fp16; host upcasts.

Per-core schedule (per batch):
    - Y8^T [e, s] via fp8 DoubleRow matmul with W stationary
    - ZT [s, e] fp16 via matmul with x stationary (chunks of 128 rows at
      stride 120 so each t-block's 128-wide window is one partition-aligned
      contraction)
    - per PAIR of 3-block groups (6 t-blocks of 120): banded fp8 Grams land in
      one [128, 768] tile; ONE pitch-768 DRAM write + stride-769 read extracts
      all 6 diagonal 9-bands; softmax (scale folded into Exp, no
      max-subtraction needed at these magnitudes); the banded matrix
      Aband^T[t, s] is built ON-CHIP by a single GPSIMD local_scatter
      (per-partition indices t + w + 128k, no DRAM round trip); PE-transpose
      to [s, t]; then F[e-chunk, t] = ZT_chunk^T @ Aband, evacuated as
      out = F/8 + bo into a two-group staging tile flushed by one output DMA
      per 720 columns.
    The emission order software-pipelines everything: gram pairs are emitted
    as soon as their Y columns exist, aggregation groups as soon as their ZT
    chunks exist, and batch b+1's x-load/projections overlap batch b's block
    phase. PSUM-evacuation work is spread across the Activation, DVE and Pool
    engines; DMA instruction count is kept low (HWDGE descriptor-gen costs
    ~625ns per DMA, serialized).
"""

import math
from contextlib import ExitStack

import numpy as np
import ml_dtypes

import concourse.bass as bass
import concourse.tile as tile
from concourse import bacc, mybir
from concourse.bass_utils import run_bass_kernel_spmd
from concourse.masks import make_identity

f32 = mybir.dt.float32
fp16 = mybir.dt.float16
fp8 = mybir.dt.float8e4
i16 = mybir.dt.int16
DR = mybir.MatmulPerfMode.DoubleRow

B, D, T = 16, 512, 2048
NCORES = 8
BPC = B // NCORES  # batches per core
PAD = 4
W = 9
S = T + 2 * PAD  # 2056
SCALE = 1.0 / (math.sqrt(D) * 1.5)
YPAD = 4  # extra left shift of the yt layout: the gram's stationary (weight)
# operand then starts at byte offset t0+8, satisfying the fp8 DoubleRow
# ldweights alignment check; SY_LEN pads S accordingly.
SY_LEN = S + 2 * YPAD

SX = 8.0  # host pre-scale on x (fp16/fp8 carry 8x)
SW = 64.0  # host pre-scale on Wq (fp8 carries 64Wq)
SY = 8.0  # scale carried by Y8 (fp8 carries 8Y)

P = 128
DC = D // P  # 4 chunks of the hidden dim
TB = 120  # t-block size; window = TB + 8 = 128 fits one contraction
NBLK = (T + TB - 1) // TB  # 18
GB = 3  # t-blocks per agg group (PSUM / output staging granularity)
NGRP = NBLK // GB  # 6
PB = 2 * GB  # t-blocks per band pair (DRAM/scatter batching)
NPAIR = NBLK // PB  # 3

REPS = 1  # device-side repeat count (timing amplification only)
UNROLL = False  # python-unroll REPS instead of tc.For_i (sim analysis only)

# DRAM scratch geometry: grams of a pair are packed [t, 128k + j] (k = block
# in pair); pitch-768 write + stride-769 read extracts the 6 diagonal bands.
G_WPITCH = PB * P  # 768
G_FLAT = (G_WPITCH + 1) * P  # write covers all 128 g_all rows

NIDX = PB * W + 2  # 56 scatter indices per partition (padded even, 2 dummies)

_S_TILES = [(0, 192), (192, 448), (640, 448), (1088, 448), (1536, 384), (1920, 136)]


def _body(nc, tc, ctx, x, x8in, wq, bq, wo, bo, sidx, y):
    singles = ctx.enter_context(tc.tile_pool(name="singles", bufs=1))

    ident_b = singles.tile([P, P], fp16)
    make_identity(nc, ident_b)

    # weights arrive pre-transposed (and pre-scaled) from the host:
    # wq is (64*Wq).T in fp8, wo is Wo.T in fp16
    wqT = singles.tile([P, DC, D], fp8)  # [d_part, d_chunk, e]
    woT = singles.tile([P, DC, D], fp16)
    nc.sync.dma_start(wqT, wq.rearrange("(c p) e -> p c e", p=P))
    nc.sync.dma_start(woT, wo.rearrange("(c p) e -> p c e", p=P))

    bq_sb = singles.tile([P, DC], f32)  # carries 8*bq
    nc.sync.dma_start(bq_sb, bq.rearrange("(c p) -> p c", p=P))
    bo_sb = singles.tile([P, DC], f32)
    nc.sync.dma_start(bo_sb, bo.rearrange("(c p) -> p c", p=P))
    sidx_sb = singles.tile([P, NIDX], i16)  # per-partition scatter indices
    nc.sync.dma_start(sidx_sb, sidx)

    # ---------------- pools ----------------
    xp_pool = ctx.enter_context(tc.tile_pool(name="xp", bufs=2))
    x8_pool = ctx.enter_context(tc.tile_pool(name="x8", bufs=2))
    y_pool = ctx.enter_context(tc.tile_pool(name="ypool", bufs=1))
    zt_pool = ctx.enter_context(tc.tile_pool(name="zt", bufs=1))
    small = ctx.enter_context(tc.tile_pool(name="small", bufs=NPAIR + 1))
    abp = ctx.enter_context(tc.tile_pool(name="abp", bufs=NBLK + 2))
    fsb_pool = ctx.enter_context(tc.tile_pool(name="fsb", bufs=2))
    proj_ps = ctx.enter_context(tc.tile_pool(name="proj_ps", bufs=3, space="PSUM"))
    gram_ps = ctx.enter_context(tc.tile_pool(name="gram_ps", bufs=2, space="PSUM"))
    tr_ps = ctx.enter_context(tc.tile_pool(name="tr_ps", bufs=1, space="PSUM"))
    f_ps = ctx.enter_context(tc.tile_pool(name="f_ps", bufs=2, space="PSUM"))
    dram_g = ctx.enter_context(tc.tile_pool(name="dram_g", bufs=NPAIR + 1, space="DRAM"))

    xps, x8s, yts, zts, pends, fstage = {}, {}, {}, {}, {}, {}

    def stage_load(bi):
        # load x (pre-scaled 8x: fp16 and fp8 copies) with zero halo
        xp = xp_pool.tile([P, DC, S], fp16, tag="xp")
        x8 = x8_pool.tile([P, DC, S], fp8, tag="x8")
        xps[bi], x8s[bi] = xp, x8
        nc.gpsimd.memset(xp[:, :, 0:PAD], 0.0)
        nc.gpsimd.memset(xp[:, :, S - PAD : S], 0.0)
        nc.gpsimd.memset(x8[:, :, 0:PAD], 0.0)
        nc.gpsimd.memset(x8[:, :, S - PAD : S], 0.0)
        xv = x[bi].rearrange("(c p) t -> p c t", p=P)
        xv8 = x8in[bi].rearrange("(c p) t -> p c t", p=P)
        # x8 feeds the Y projection, which runs first: load it before xp,
        # first halves first so the first Y tiles start sooner
        for t0, t1 in ((0, T // 2), (T // 2, T)):
            for dc in range(DC):
                nc.sync.dma_start(
                    out=x8[:, dc, PAD + t0 : PAD + t1], in_=xv8[:, dc, t0:t1]
                )
        for dc in range(DC):
            nc.sync.dma_start(out=xp[:, dc, PAD : PAD + T], in_=xv[:, dc, :])

    def stage_y_gram(bi):
        # Y8^T = fp8(8*(Wq @ x + bq)) [e_part, e_chunk, s], s-tile-major via
        # DoubleRow fp8 matmuls; a band pair is emitted as soon as the Y
        # columns its windows need are complete.
        x8 = x8s[bi]
        yt = y_pool.tile([P, DC, SY_LEN], fp8, tag="yt")
        yts[bi] = yt
        pends[bi] = {}
        done = 0
        for s0, sn in _S_TILES:
            for ec in range(DC):
                ps = proj_ps.tile([P, 512], f32, tag="proj")
                for i in range(2):
                    nc.tensor.matmul(
                        ps[:, 0:sn],
                        wqT[:, 2 * i : 2 * i + 2, ec * P : (ec + 1) * P],
                        x8[:, 2 * i : 2 * i + 2, s0 : s0 + sn],
                        start=(i == 0),
                        stop=(i == 1),
                        perf_mode=DR,
                    )
                # Y8 = PSUM/64 + 8bq  (PSUM = 512*(Y - bq))
                nc.scalar.activation(
                    out=yt[:, ec, YPAD + s0 : YPAD + s0 + sn],
                    in_=ps[:, 0:sn],
                    func=mybir.ActivationFunctionType.Identity,
                    bias=bq_sb[:, ec : ec + 1],
                    scale=1.0 / SW,
                )
            avail = s0 + sn
            # a pair's reads extend to the M-padded lhsT end of its last
            # block: TB*(PB*done + PB - 1) + PAD + 128
            while done < NPAIR and min(
                TB * (PB * done + PB - 1) + PAD + P, S
            ) <= avail:
                band_pair(bi, done)
                done += 1
        while done < NPAIR:
            band_pair(bi, done)
            done += 1

    def band_pair(bi, pi):
        # 6 banded fp8 grams -> one DRAM pitch-trick round trip for the diag
        # bands -> softmax -> banded matrix via gpsimd local_scatter (on-chip)
        # -> PE transposes
        yt = yts[bi]
        pend = pends[bi]
        blocks = []
        for k in range(PB):
            t0 = TB * (pi * PB + k)
            tw = min(TB, T - t0)
            blocks.append((t0, tw))
        full = all(tw == TB for _, tw in blocks)

        g_all = small.tile([P, G_WPITCH], fp16, tag="gall")
        if not full:
            nc.vector.memset(g_all, 0.0)
        # 4 gram blocks share one 2KB PSUM bank -> one DVE copy per tile.
        # The stationary operand is padded to 128 columns when in range so the
        # fast-weight-load path kicks in; all 128 gram rows are copied so the
        # diag read of rows 120..127 sees finite (if unused) data.
        for h in range(2):
            g_ps = gram_ps.tile([P, 4, P], f32, tag="gram")
            nsl = 4 if h == 0 else 2
            for sl in range(nsl):
                k = h * 4 + sl
                t0, tw = blocks[k]
                sw = tw + 2 * PAD
                mw = min(P, S - (t0 + PAD))
                for i in range(2):
                    nc.tensor.matmul(
                        g_ps[0:mw, sl, 0:sw],
                        yt[:, 2 * i : 2 * i + 2, YPAD + t0 + PAD : YPAD + t0 + PAD + mw],
                        yt[:, 2 * i : 2 * i + 2, YPAD + t0 : YPAD + t0 + sw],
                        start=(i == 0),
                        stop=(i == 1),
                        perf_mode=DR,
                    )
            if full:
                nc.vector.tensor_copy(
                    out=g_all[:, h * 4 * P : (h * 4 + nsl) * P],
                    in_=g_ps[:, 0:nsl, :],
                )
            else:
                for sl in range(nsl):
                    k = h * 4 + sl
                    t0, tw = blocks[k]
                    sw = tw + 2 * PAD
                    mw = min(P, S - (t0 + PAD))
                    nc.vector.tensor_copy(
                        out=g_all[0:mw, k * P : k * P + sw],
                        in_=g_ps[0:mw, sl, 0:sw],
                    )

        gflat = dram_g.tile([G_FLAT], fp16, tag="gflat")
        gw = bass.AP(
            tensor=gflat.tensor,
            offset=gflat.offset,
            ap=[[G_WPITCH, P], [1, G_WPITCH]],
        )
        nc.sync.dma_start(gw, g_all)
        # diag bands of all 6 grams: elem [t, k, w] = flat[(GP+1)t + 128k + w]
        gr = bass.AP(
            tensor=gflat.tensor,
            offset=gflat.offset,
            ap=[[G_WPITCH + 1, TB], [P, PB], [1, W]],
        )
        e_all = small.tile([TB, PB, W], fp16, tag="eall")
        with nc.allow_non_contiguous_dma(reason="diag band read"):
            nc.sync.dma_start(e_all, gr)

        # softmax over the 9-wide window (values are small; no max-sub).
        # gram PSUM carries 64*E, folded into the Exp scale.
        eexp = small.tile([TB, PB, W], f32, tag="eexp")
        nc.scalar.activation(
            out=eexp,
            in_=e_all,
            func=mybir.ActivationFunctionType.Exp,
            scale=SCALE / (SY * SY),
        )
        ssum = small.tile([TB, PB], f32, tag="ssum")
        nc.vector.reduce_sum(out=ssum, in_=eexp, axis=mybir.AxisListType.X)
        nc.vector.reciprocal(out=ssum, in_=ssum)
        attn = small.tile([P, NIDX], fp16, tag="attn")
        # rows 120..127 and the two pad columns are never scattered (idx -1)
        # but must be finite; zero the whole tile first
        nc.vector.memset(attn, 0.0)
        for k in range(PB):
            nc.vector.tensor_scalar_mul(
                attn[0:TB, k * W : (k + 1) * W],
                eexp[:, k, :],
                ssum[:, k : k + 1],
            )

        # banded matrix Ab^T[t, 128k + s] = attn[t, k, s - t] built on-chip
        abts = small.tile([P, PB * P], fp16, tag="abts")
        nc.gpsimd.local_scatter(
            abts, attn, sidx_sb, channels=P, num_elems=PB * P, num_idxs=NIDX
        )
        for k, (t0, tw) in enumerate(blocks):
            trp = tr_ps.tile([P, TB], fp16, tag="trp")
            nc.tensor.transpose(
                trp, abts[0:TB, k * P : (k + 1) * P], ident_b[0:TB, 0:TB]
            )
            ab = abp.tile([P, TB], fp16, tag="ab")
            nc.vector.tensor_copy(out=ab, in_=trp)
            pend.setdefault((pi * PB + k) // GB, []).append((ab, t0, tw))

    def stage_z_agg(bi):
        # ZT = (Wo @ xp)^T   [s_part, block, e] fp16 carrying 8Z; an agg
        # group is emitted as soon as its GB chunks of ZT are in place.
        xp = xps[bi]
        zt = zt_pool.tile([P, NBLK, D], fp16, tag="zt")
        zts[bi] = zt
        for ib in range(NBLK):
            s0z = TB * ib
            snz = min(P, S - s0z)
            ps = proj_ps.tile([P, 512], f32, tag="proj")
            for dc in range(DC):
                nc.tensor.matmul(
                    ps[0:snz, :],
                    xp[:, dc, s0z : s0z + snz],
                    woT[:, dc, :],
                    start=(dc == 0),
                    stop=(dc == DC - 1),
                )
            if snz < P:
                nc.gpsimd.memset(zt[:, ib, :], 0.0)
            # Pool/GPSIMD cannot read PSUM; alternate the two evac engines
            if ib % 2 == 0:
                nc.vector.tensor_copy(out=zt[0:snz, ib, :], in_=ps[0:snz, :])
            else:
                nc.scalar.copy(out=zt[0:snz, ib, :], in_=ps[0:snz, :])
            if ib % GB == GB - 1:
                agg_group(bi, ib // GB)

    def agg_group(bi, gi):
        # F[e-chunk, t] = ZT_chunk^T @ Aband (= 8*out); evacuate as
        # out = F/8 + bo into the two-group staging tile, flushed by one
        # output DMA per pair of groups.
        zt = zts[bi]
        abs_ = pends[bi].pop(gi)
        half = gi % 2
        if half == 0:
            f_new = fsb_pool.tile([P, DC, 2 * GB * TB], fp16, tag="fall")
            fstage[bi] = f_new
        f_all = fstage[bi]
        hoff = half * GB * TB
        for ec in range(DC):
            f_psum = f_ps.tile([P, GB * TB], f32, tag="fps")
            for g, (ab, t0, tw) in enumerate(abs_):
                jb = gi * GB + g
                nc.tensor.matmul(
                    f_psum[:, g * TB : g * TB + tw],
                    zt[:, jb, ec * P : (ec + 1) * P],
                    ab[:, 0:tw],
                    start=True,
                    stop=True,
                )
            full = all(tw == TB for _, _, tw in abs_)
            spans = (
                [(0, GB * TB)]
                if full
                else [(g * TB, g * TB + tw) for g, (_, _, tw) in enumerate(abs_)]
            )
            for lo, hi in spans:
                dst = f_all[:, ec, hoff + lo : hoff + hi]
                src = f_psum[:, lo:hi]
                if ec < 2:
                    nc.scalar.activation(
                        out=dst,
                        in_=src,
                        func=mybir.ActivationFunctionType.Identity,
                        bias=bo_sb[:, ec : ec + 1],
                        scale=1.0 / SX,
                    )
                else:
                    nc.vector.tensor_scalar(
                        out=dst,
                        in0=src,
                        scalar1=1.0 / SX,
                        scalar2=bo_sb[:, ec : ec + 1],
                        op0=mybir.AluOpType.mult,
                        op1=mybir.AluOpType.add,
                    )
        if half == 1:
            tg0 = TB * GB * (gi - 1)
            ext = min(TB * GB * (gi + 1), T) - tg0
            yv = y[bi].rearrange("(c p) t -> p c t", p=P)[:, :, tg0 : tg0 + ext]
            nc.sync.dma_start(yv, f_all[:, :, 0:ext])

    # software pipeline: load(0) YG(0) load(1) ZA(0) YG(1) ZA(1) ...
    def pipeline(_i=None):
        for bi in range(BPC):
            stage_load(bi)
        stage_y_gram(0)
        for bi in range(1, BPC):
            stage_z_agg(bi - 1)
            stage_y_gram(bi)
        stage_z_agg(BPC - 1)

    if REPS == 1:
        pipeline()
    elif UNROLL:
        for _ in range(REPS):
            pipeline()
    else:
        with tc.For_i(0, REPS, 1):
            pipeline()


def build_nc():
    nc = bacc.Bacc("TRN2", debug=False)
    x_in = nc.dram_tensor("x", [BPC, D, T], fp16, kind="ExternalInput")
    x8_in = nc.dram_tensor("x8", [BPC, D, T], fp8, kind="ExternalInput")
    wq_in = nc.dram_tensor("WqT", [D, D], fp8, kind="ExternalInput")
    bq_in = nc.dram_tensor("bq", [D], f32, kind="ExternalInput")
    wo_in = nc.dram_tensor("WoT", [D, D], fp16, kind="ExternalInput")
    bo_in = nc.dram_tensor("bo", [D], f32, kind="ExternalInput")
    sidx_in = nc.dram_tensor("sidx", [P, NIDX], i16, kind="ExternalInput")
    y_out = nc.dram_tensor("y", [BPC, D, T], fp16, kind="ExternalOutput")

    with tile.TileContext(nc) as tc, ExitStack() as ctx:
        _body(
            nc,
            tc,
            ctx,
            x_in.ap(),
            x8_in.ap(),
            wq_in.ap(),
            bq_in.ap(),
            wo_in.ap(),
            bo_in.ap(),
            sidx_in.ap(),
            y_out.ap(),
        )
    nc.compile()
    return nc


_NC_CACHE = []


def _get_nc():
    if not _NC_CACHE:
        _NC_CACHE.append(build_nc())
    return _NC_CACHE[0]


def _scatter_idx():
    idx = np.full((P, NIDX), -1, np.int16)
    tau = np.arange(TB)[:, None]
    for k in range(PB):
        for w_ in range(W):
            idx[0:TB, k * W + w_ : k * W + w_ + 1] = P * k + tau + w_
    return idx


def _in_maps(x, Wq, bq, Wo, bo):
    xs = np.asarray(x, dtype=np.float32) * SX
    x16 = np.ascontiguousarray(xs.astype(np.float16))
    x8 = np.ascontiguousarray(xs.astype(ml_dtypes.float8_e4m3))
    WqT = np.ascontiguousarray(
        (np.asarray(Wq, dtype=np.float32).T * SW).astype(ml_dtypes.float8_e4m3)
    )
    bq = np.ascontiguousarray(np.asarray(bq, dtype=np.float32) * SY)
    WoT = np.ascontiguousarray(np.asarray(Wo, dtype=np.float32).T.astype(np.float16))
    bo = np.ascontiguousarray(np.asarray(bo, dtype=np.float32))
    sidx = _scatter_idx()
    return [
        {
            "x": x16[c * BPC : (c + 1) * BPC],
            "x8": x8[c * BPC : (c + 1) * BPC],
            "WqT": WqT,
            "bq": bq,
            "WoT": WoT,
            "bo": bo,
            "sidx": sidx,
        }
        for c in range(NCORES)
    ]


def run(trace=False, **inputs):
    nc = _get_nc()
    res = run_bass_kernel_spmd(
        nc, _in_maps(**inputs), core_ids=list(range(NCORES)), trace=trace
    )
    out = np.concatenate([r["y"] for r in res.results], axis=0).astype(np.float32)
    return out, res


def kernel(x, Wq, bq, Wo, bo):
    out, _ = run(x=x, Wq=Wq, bq=bq, Wo=Wo, bo=bo)
    return out
